# revision 6
# baseline (speedup 1.0000x reference)
"""DisMax loss first part: logits = -(|s|*d + mean_c(|s|*d)) / temp, where
d[b,c] = ||fn_b - pn_c|| / sqrt(2) = sqrt(1 - cos(f_b, p_c)) for l2-normalized rows.

Data-parallel over the batch across 8 NeuronCores; each core computes
[1024, 512] x [512, 10000] with all numerics on device (host does layout
transposes and the final bf16->f32 upcast only).

v2 redesign (from trace analysis of the gram-based baseline, 256 us):
  - The f32r per-128-block Gram matmuls ran in fp32 HIGH mode (~2 cyc/row +
    unhidden LDWEIGHTS) and dominated PE time; norms are now computed as
    square (elementwise, bf16 out) + ones-vector reduction matmuls (bf16,
    1 cyc/row), spreading the square pass across DVE/ACT/GPSIMD.
  - 1/sqrt uses DVE reciprocal_approx_fast on the [1, W] norm^2 row then an
    ACT Sqrt; Abs/Square/Sqrt all live in the one 'sqrt_and_others' table
    set, so the ACT never reloads tables.
  - PSUM/ACT main-loop groups are 2000 cols (4 banks) to amortize ACT's
    ~352-cycle fixed overhead and accumulator reads.
  - Main GEMM chunks are interleaved chunk-major across a 3-tile window
    during prototype prep so the PE never idles past the ~3.4us HAM window
    (the baseline oscillated between 1.2 and 2.4 GHz for its first 155 us).
"""

import sys
import types

for _p in ("/opt/trn_rl_repo", "/root/.axon_site"):
    if _p not in sys.path:
        sys.path.insert(0, _p)

# The NTFF profiling hook module is absent from this image's antenv package;
# inject the ctypes-based equivalent so trace=True works when requested.
if "antenv.axon_hooks" not in sys.modules:
    try:
        import trn_agent_boot.trn_boot as _tb

        _hook = _tb._ntff_profile_via_ctypes("/opt/axon/libaxon_pjrt.so")
        _m = types.ModuleType("antenv.axon_hooks")
        _m.get_axon_ntff_profile_hook = lambda: _hook
        sys.modules["antenv.axon_hooks"] = _m
    except Exception:
        pass

import numpy as np

import concourse.bacc as bacc
import concourse.tile as tile
import concourse.mybir as mybir
from concourse.bass_utils import run_bass_kernel_spmd

F32 = mybir.dt.float32
BF16 = mybir.dt.bfloat16
FP8 = mybir.dt.float8e4
ALU = mybir.AluOpType
ACTF = mybir.ActivationFunctionType
DR = mybir.MatmulPerfMode.DoubleRow

N_CORES = 8
B, C, D = 8192, 10000, 512
BPC = B // N_CORES          # 1024 batch rows per core
NB = BPC // 128             # 8 batch tiles
ND = D // 128               # 4 contraction sub-tiles
SEGW = 1000                 # prototype prep segment width
NSEG = C // SEGW            # 10
GW = 2000                   # main GEMM/ACT column group (4 PSUM banks)
NG = C // GW                # 5
OBW = 2500                  # affine/store chunk
NOB = C // OBW              # 4
WIN = 3                     # batch tiles interleaved with prep

# prep engine assignment per segment (tuned via trace measurements).
# GPSIMD cannot touch PSUM, so the normalize (reads the PSUM broadcast)
# stays on DVE; the square pass is spread over ACT/GPSIMD to keep DVE free.
SQ_ENG = ["act", "gps", "act", "gps", "act", "gps", "act", "gps", "gps", "gps"]
NORM_ENG = ["dve"] * NSEG
AFF_ENG = ["gps"] * (NB - 1) + ["dve"]  # last tile probes DVE tensor_scalar rate


def build_nc():
    nc = bacc.Bacc("TRN2", target_bir_lowering=False, debug=False,
                   num_devices=N_CORES)
    ft_h = nc.dram_tensor("ft", [D, BPC], F32, kind="ExternalInput")
    pt_h = nc.dram_tensor("pt", [D, C], F32, kind="ExternalInput")
    s_h = nc.dram_tensor("s", [1, 2], F32, kind="ExternalInput")
    o_h = nc.dram_tensor("o", [BPC, C], BF16, kind="ExternalOutput")

    from contextlib import ExitStack

    with tile.TileContext(nc) as tc:
        with ExitStack() as stack:
            ep = stack.enter_context
            const_pool = ep(tc.tile_pool(name="const", bufs=1))
            persist_pool = ep(tc.tile_pool(name="persist", bufs=1))
            fstage_pool = ep(tc.tile_pool(name="fstage", bufs=1))
            pstage_pool = ep(tc.tile_pool(name="pstage", bufs=2))
            sq_pool = ep(tc.tile_pool(name="sq", bufs=2))
            row_pool = ep(tc.tile_pool(name="rows", bufs=1))
            dq_pool = ep(tc.tile_pool(name="dq", bufs=WIN))
            rs_pool = ep(tc.tile_pool(name="rs", bufs=WIN))
            tail_pool = ep(tc.tile_pool(name="tail", bufs=2))
            ob_pool = ep(tc.tile_pool(name="ob", bufs=3))
            # one shared PSUM pool: 2 x [128, 4, 512] f32 = all 8 banks.
            # main groups use all 4 banks; prep tiles use banks 0-1 for the
            # [1, 2, 500] norm^2 reduction rows and banks 2-3 for the
            # [128, 2, 500] inv-norm broadcast.
            ps_pool = ep(tc.tile_pool(name="ps", bufs=2, space="PSUM"))

            # persistent fp8 normalized, transposed operands
            pnT = persist_pool.tile([128, ND, C], FP8, tag="pnT")
            fnT = persist_pool.tile([128, ND, BPC], FP8, tag="fnT")
            cb = persist_pool.tile([128, 2], F32, tag="cb")  # c0, c1

            ones_f = const_pool.tile([1, 128], F32, tag="ones_f")
            nc.vector.memset(ones_f[:, :], 1.0)
            ones_row = const_pool.tile([1, 128], BF16, tag="ones_row")
            nc.vector.memset(ones_row[:, :], 1.0)
            ones_col = const_pool.tile([128, 1], BF16, tag="ones_col")
            nc.vector.memset(ones_col[:, :], 1.0)

            # ---- scalar params: c0 = -|ds|/temp, c1 = c0/C ----------------
            stile = const_pool.tile([1, 2], F32, tag="stile")
            nc.sync.dma_start(stile[:, :], s_h[:, :])
            cv = const_pool.tile([1, 2], F32, tag="cvals")
            tmp = const_pool.tile([1, 2], F32, tag="scaltmp")
            nc.scalar.activation(tmp[:, 0:1], stile[:, 0:1], ACTF.Abs)
            nc.vector.reciprocal(tmp[:, 1:2], stile[:, 1:2])
            nc.vector.scalar_tensor_tensor(cv[:, 0:1], tmp[:, 0:1], -1.0,
                                           tmp[:, 1:2], op0=ALU.mult,
                                           op1=ALU.mult)
            nc.vector.tensor_scalar(cv[:, 1:2], cv[:, 0:1], 1.0 / C, None,
                                    op0=ALU.mult)
            ps_b = ps_pool.tile([128, 4, 512], F32, tag="ps", name="cbb")
            nc.tensor.matmul(ps_b[:, 0, :2], ones_f[:, :], cv[:, :],
                             start=True, stop=True)
            nc.vector.tensor_copy(cb[:, :], ps_b[:, 0, :2])

            ft_r = ft_h[:, :].rearrange("(t p) b -> p t b", p=128)
            pt_r = pt_h[:, :].rearrange("(t p) c -> p t c", p=128)

            def norm_bcast(ps, sqt, W, nh):
                """sq [128, ND, W] -> inv-norm broadcast in ps[:, 2:4, :nh].

                ps banks 0-1: norm^2 rows [1, 2, nh]; banks 2-3: broadcast.
                """
                for h in range(2):
                    for d in range(ND):
                        nc.tensor.matmul(ps[0:1, h, :nh], ones_col[:, :],
                                         sqt[:, d, h * nh:(h + 1) * nh],
                                         start=(d == 0), stop=(d == ND - 1))
                irow = row_pool.tile([1, 2, 512], F32, tag="irow",
                                     name=f"irow_{W}_{id(sqt)}")
                nc.vector.reciprocal_approx_fast(irow[:, :, :nh],
                                                 ps[0:1, 0:2, :nh])
                hrow = row_pool.tile([1, 2, 512], BF16, tag="hrow",
                                     name=f"hrow_{W}_{id(sqt)}")
                nc.scalar.activation(hrow[:, :, :nh], irow[:, :, :nh],
                                     ACTF.Sqrt)
                for h in range(2):
                    nc.tensor.matmul(ps[:, 2 + h, :nh], ones_row[:, :],
                                     hrow[:, h, :nh], start=True, stop=True)

            # ---- features: load, norms, normalize to fp8 ------------------
            fstage = fstage_pool.tile([128, ND, BPC], F32, tag="fst")
            nc.sync.dma_start(fstage[:, :, :], ft_r[:, :, :])
            # features squared: [128, ND, 1024] bf16 via one DVE op
            sqf = sq_pool.tile([128, ND, 1024], BF16, tag="sqf")
            nc.vector.tensor_tensor(
                sqf[:, :, :], fstage[:, :, :], fstage[:, :, :], op=ALU.mult)
            psf = ps_pool.tile([128, 4, 512], F32, tag="ps", name="psf")
            norm_bcast(psf, sqf, 1024, 512)
            for d in range(ND):
                nc.vector.tensor_tensor(
                    fnT[:, d, :].rearrange("p (h c) -> p h c", h=2),
                    fstage[:, d, :].rearrange("p (h c) -> p h c", h=2),
                    psf[:, 2:4, :512], op=ALU.mult)

            # ---- prototype segments + interleaved main groups -------------
            def seg_prep(s):
                pst = pstage_pool.tile([128, ND, SEGW], F32, tag="pst",
                                       name=f"pst_{s}")
                nc.sync.dma_start(pst[:, :, :],
                                  pt_r[:, :, s * SEGW:(s + 1) * SEGW])
                sqt = sq_pool.tile([128, ND, SEGW], BF16, tag="sq",
                                   name=f"sq_{s}")
                if SQ_ENG[s] == "dve":
                    nc.vector.tensor_tensor(sqt[:, :, :], pst[:, :, :],
                                            pst[:, :, :], op=ALU.mult)
                elif SQ_ENG[s] == "act":
                    nc.scalar.activation(sqt[:, :, :], pst[:, :, :],
                                         ACTF.Square)
                else:
                    nc.gpsimd.tensor_tensor(sqt[:, :, :], pst[:, :, :],
                                            pst[:, :, :], op=ALU.mult)
                ps = ps_pool.tile([128, 4, 512], F32, tag="ps",
                                  name=f"psn_{s}")
                norm_bcast(ps, sqt, SEGW, 500)
                eng = nc.vector if NORM_ENG[s] == "dve" else nc.gpsimd
                for d in range(ND):
                    eng.tensor_tensor(
                        pnT[:, d, s * SEGW:(s + 1) * SEGW].rearrange(
                            "p (h c) -> p h c", h=2),
                        pst[:, d, :].rearrange("p (h c) -> p h c", h=2),
                        ps[:, 2:4, :500], op=ALU.mult)

            def main_group(t, g, rs, dq):
                """DR GEMM + fused sqrt for cols [g*GW, (g+1)*GW), tile t."""
                pm = ps_pool.tile([128, 4, 512], F32, tag="ps",
                                  name=f"pm_{t}_{g}")
                for sub in range(4):
                    c0 = g * GW + sub * 500
                    for dp in range(ND // 2):
                        nc.tensor.matmul(
                            pm[:, sub, :500],
                            fnT[:, 2 * dp:2 * dp + 2,
                                t * 128:(t + 1) * 128],
                            pnT[:, 2 * dp:2 * dp + 2, c0:c0 + 500],
                            start=(dp == 0), stop=(dp == ND // 2 - 1),
                            perf_mode=DR)
                nc.scalar.activation(
                    dq[:, g * GW:(g + 1) * GW].rearrange(
                        "p (a c) -> p a c", a=4),
                    pm[:, :, :500], ACTF.Sqrt, bias=1.0, scale=-1.0,
                    accum_out=rs[:, g:g + 1])

            def tail(t, rs, dq):
                rsum = tail_pool.tile([128, 1], F32, tag="rsum",
                                      name=f"rsum_{t}")
                bvec = tail_pool.tile([128, 1], F32, tag="bvec",
                                      name=f"bvec_{t}")
                nc.vector.reduce_sum(rsum[:, :], rs[:, :],
                                     axis=mybir.AxisListType.X)
                nc.vector.tensor_scalar(bvec[:, :], rsum[:, :], cb[:, 1:2],
                                        None, op0=ALU.mult)
                eng = nc.gpsimd if AFF_ENG[t] == "gps" else nc.vector
                for q in range(NOB):
                    ob = ob_pool.tile([128, OBW], BF16, tag="ob",
                                      name=f"ob_{t}_{q}")
                    eng.tensor_scalar(ob[:, :],
                                      dq[:, q * OBW:(q + 1) * OBW],
                                      cb[:, 0:1], bvec[:, 0:1],
                                      op0=ALU.mult, op1=ALU.add)
                    nc.sync.dma_start(
                        o_h[t * 128:(t + 1) * 128, q * OBW:(q + 1) * OBW],
                        ob[:, :])

            dqs = {}
            rss = {}
            for t in range(WIN):
                rss[t] = rs_pool.tile([128, NG], F32, tag="rs",
                                      name=f"rs_{t}")
                dqs[t] = dq_pool.tile([128, C], BF16, tag="dq",
                                      name=f"dq_{t}")
            for s in range(NSEG):
                seg_prep(s)
                if s % 2 == 1:
                    g = (s - 1) // 2
                    for t in range(WIN):
                        main_group(t, g, rss[t], dqs[t])
            for t in range(WIN):
                tail(t, rss[t], dqs[t])
            for t in range(WIN, NB):
                rs = rs_pool.tile([128, NG], F32, tag="rs", name=f"rs_{t}")
                dq = dq_pool.tile([128, C], BF16, tag="dq", name=f"dq_{t}")
                for g in range(NG):
                    main_group(t, g, rs, dq)
                tail(t, rs, dq)

    nc.compile()
    return nc


_CACHE = {}


def _get_nc():
    if "nc" not in _CACHE:
        _CACHE["nc"] = build_nc()
    return _CACHE["nc"]


def make_in_maps(features, prototypes, distance_scale, temperature):
    f = np.asarray(features, dtype=np.float32)
    ft = np.ascontiguousarray(f.T)              # [D, B]
    pt = np.ascontiguousarray(
        np.asarray(prototypes, dtype=np.float32).T)  # [D, C]
    s = np.array([[np.float32(np.asarray(distance_scale).reshape(-1)[0]),
                   np.float32(np.asarray(temperature).reshape(-1)[0])]],
                 dtype=np.float32)
    return [
        {"ft": np.ascontiguousarray(ft[:, i * BPC:(i + 1) * BPC]),
         "pt": pt, "s": s}
        for i in range(N_CORES)
    ]


def run(features, prototypes, distance_scale, temperature, **kwargs):
    nc = _get_nc()
    in_maps = make_in_maps(features, prototypes, distance_scale, temperature)
    res = run_bass_kernel_spmd(nc, in_maps, core_ids=list(range(N_CORES)),
                               **kwargs)
    out = np.concatenate(
        [np.asarray(res.results[i]["o"]).astype(np.float32)
         for i in range(N_CORES)], axis=0)
    return out, res


def kernel(features, prototypes, distance_scale, temperature):
    out, _ = run(features, prototypes, distance_scale, temperature)
    return out
